# revision 1
# baseline (speedup 1.0000x reference)
"""Trainium2 Bass kernel for nn_Attention: full attention layer
(QKV proj + per-head RMSNorm on q,k + softmax attention + out proj),
data-parallel over batch across 8 NeuronCores (2 batch elems per core).

Per-core dataflow (bf16 compute, f32 PSUM/stats):
  A. x [tok, hid] tiles -> f32 DMA on hardware DGE queues -> DVE cast to
     bf16 -> PE transpose -> xT [hid, tok].
  B. QKV proj in layout [tok, outdim]: lhsT = xT tiles (stationary),
     rhs = w_qkv.T chunks (head-aligned widths 432/432/288); bias added
     during the DVE PSUM evacuation. q,k stored bf16 per token-tile; v
     lands in v_aug tiles [128, 16*97] where each head owns 97 cols:
     72 v | 24 zero | 1 one.
  C. RMSNorm: ACT Square + DVE reduce -> sumsq (one stats tile per batch);
     one ACT Sqrt + one DVE reciprocal_approx_fast per batch; applied
     in-place via broadcast-AP multiply. gamma_q*gamma_k folds into kT.
  D. Attention per head, scores TRANSPOSED: sT[j,i] = kT_j.T @ qT so the
     softmax axis is on partitions; exp on ScalarE over [128, 1024] psum
     pairs (no max subtraction: |logit| <= sqrt(72) after RMSNorm);
     PV lhsT = v_aug head slice [128, 97] -> accumulator row 96 is the
     softmax denominator. Per-head normalize chain with two DMA hops:
     the denominator row cast-DMAs to f32 DRAM, broadcast-reads back as
     [72, S], reciprocal_approx_fast runs on the broadcast, one multiply
     normalizes posb in place, and partition-shifting SBUF->SBUF DMAs
     repack head h into DENSE attn rows 72h..72h+71 (9 out-proj K-tiles,
     no zero padding). The LAST head (the exposed critical path into the
     final out-proj) instead broadcasts its denominator on the PE -- a
     K=1 matmul at tile_position (96,0) into the freed PV psum slot --
     skipping both DMA hops. Scores are software-pipelined one jt ahead; the
     next head's PE transposes and the next batch's x-phase interleave
     into the loop so the in-order PE stream stays HAM-warm. Attention
     is ScalarE(exp)-limited at ~1.07us/jt; the PE slack absorbs the
     transposes.
  E. Out proj (dense 9 K-tiles, bf16 w_proj.T + f32 bias), emitted AFTER
     the next batch's QKV phase so the last heads' normalize-chain tail
     hides under independent PE work; all three weight chunks prefetch
     at phase start and 4 output buffers ride out the out-DMA latency.

Measured on trn2 (8 cores, axon): ~648 us HW exec (vs 779 us for the
session-start version on the same device at 2.4 GHz; PE 88.9% busy),
rel err 6.1e-3. Note the chip sometimes drops to 2.0 GHz (P0 power
state), scaling all engine times by ~1.2x run-to-run.
"""
import sys
import numpy as np

sys.path.insert(0, "/opt/trn_rl_repo")

import concourse.bass as bass  # noqa: E402,F401
import concourse.tile as tile  # noqa: E402
import concourse.mybir as mybir  # noqa: E402
from concourse import bacc  # noqa: E402
from concourse.bass_utils import run_bass_kernel_spmd  # noqa: E402
from concourse.masks import make_identity  # noqa: E402
import ml_dtypes  # noqa: E402

F32 = mybir.dt.float32
F32R = mybir.dt.float32r
BF16 = mybir.dt.bfloat16
AF = mybir.ActivationFunctionType
MUL = mybir.AluOpType.mult
ADD = mybir.AluOpType.add

B, S, H = 16, 1024, 1152
NH, HD = 16, 72
B_LOCAL = 2
N_CORES = 8
TT = S // 128             # 8 token tiles per batch
CHUNKS = [(0, 432), (432, 432), (864, 288)]   # head-aligned proj chunks
KT_O = H // 128           # 9 K-tiles for out proj (dense attn rows)
NP = 384                  # out-proj N chunk
SCALE = 1.0 / float(np.sqrt(HD))
EPS = float(np.finfo(np.float32).eps)


def build_nc(n_batch=B_LOCAL):
    nc = bacc.Bacc("TRN2", target_bir_lowering=False, debug=False,
                   num_devices=N_CORES)
    x_d = nc.dram_tensor("x", [n_batch, S, H], F32, kind="ExternalInput").ap()
    wqkv_d = nc.dram_tensor("wqkvt", [H, 3 * H], BF16, kind="ExternalInput").ap()
    bias_d = nc.dram_tensor("biasb", [128, 3 * H], BF16, kind="ExternalInput").ap()
    gqk_d = nc.dram_tensor("gqk", [HD, 1], F32, kind="ExternalInput").ap()
    wp_d = nc.dram_tensor("wprojt", [H, H], BF16, kind="ExternalInput").ap()
    bp_d = nc.dram_tensor("bprojb", [128, H], F32, kind="ExternalInput").ap()
    out_d = nc.dram_tensor("out", [n_batch, S, H], F32, kind="ExternalOutput").ap()

    with tile.TileContext(nc) as tc:
        _build(nc, tc, n_batch, x_d, wqkv_d, bias_d, gqk_d, wp_d, bp_d, out_d)
    nc.compile()
    return nc


def _build(nc, tc, n_batch, x_d, wqkv_d, bias_d, gqk_d, wp_d, bp_d, out_d):
    import contextlib
    ctx = contextlib.ExitStack()
    with ctx:
        sbc = ctx.enter_context(tc.tile_pool(name="const", bufs=1))
        sbx = ctx.enter_context(tc.tile_pool(name="sbx", bufs=1))
        sbqk = ctx.enter_context(tc.tile_pool(name="sbqk", bufs=1))
        sbv = ctx.enter_context(tc.tile_pool(name="sbv", bufs=1))
        sba = ctx.enter_context(tc.tile_pool(name="sba", bufs=1))
        sbw = ctx.enter_context(tc.tile_pool(name="sbw", bufs=2))
        sbwp = ctx.enter_context(tc.tile_pool(name="sbwp", bufs=3))
        sbxf = ctx.enter_context(tc.tile_pool(name="sbxf", bufs=2))
        sbyo = ctx.enter_context(tc.tile_pool(name="sbyo", bufs=4))
        sbt = ctx.enter_context(tc.tile_pool(name="sbt", bufs=2))
        sbqt = ctx.enter_context(tc.tile_pool(name="sbqt", bufs=2))
        sbs = ctx.enter_context(tc.tile_pool(name="sbs", bufs=1))
        sbr = ctx.enter_context(tc.tile_pool(name="sbr", bufs=2))
        sbpo = ctx.enter_context(tc.tile_pool(name="sbpo", bufs=1))
        sbe = ctx.enter_context(tc.tile_pool(name="sbe", bufs=3))
        sbrc = ctx.enter_context(tc.tile_pool(name="sbrc", bufs=1))
        dpool = ctx.enter_context(tc.tile_pool(name="dram", bufs=2, space="DRAM"))
        ps_s = ctx.enter_context(tc.tile_pool(name="pss", bufs=2, space="PSUM"))
        ps_sc = ctx.enter_context(tc.tile_pool(name="pssc", bufs=2, space="PSUM"))
        ps_pv = ctx.enter_context(tc.tile_pool(name="pspv", bufs=1, space="PSUM"))

        # constants
        id32 = sbc.tile([128, 128], F32)
        make_identity(nc, id32[:])
        id16 = sbc.tile([128, 128], BF16)
        nc.vector.tensor_copy(id16[:], id32[:])
        bias_b = sbc.tile([128, 3 * H], BF16)
        nc.sync.dma_start(bias_b[:], bias_d[:])
        zo = sbc.tile([128, 25], F32)          # vaug pad+ones template
        nc.vector.memset(zo[:, 0:24], 0.0)
        nc.vector.memset(zo[:, 24:25], 1.0)
        eps_t = sbc.tile([128, 1], F32)
        nc.vector.memset(eps_t[:], EPS)
        gqk = sbc.tile([HD, 1], F32)
        nc.sync.dma_start(gqk[:], gqk_d[:])
        ones72 = sbc.tile([128, HD], BF16)   # K=1 PE-broadcast stationary
        nc.vector.memset(ones72[:], 1.0)
        bp_b = sbc.tile([128, H], F32)
        nc.sync.dma_start(bp_b[:], bp_d[:])

        def phase_a_tile(b, xTv, m):
            # f32 DMA on the fast hardware DGE queues, cast on DVE (the
            # gpsimd software-DGE cast-DMA dispatches too slowly); DMA,
            # cast and transpose pipeline per 384-col third of the tile
            xf = sbxf.tile([128, H], F32, tag="xf", name=f"xf_{b}_{m}")
            xc = sbt.tile([128, H], BF16, tag="xc", name=f"xc_{b}_{m}")
            for g in range(3):  # 3 k-blocks per psum group
                sl = slice(384 * g, 384 * (g + 1))
                nc.sync.dma_start(xf[:, sl], x_d[b, 128 * m:128 * (m + 1), sl])
                nc.vector.tensor_copy(xc[:, sl], xf[:, sl])
                pst = ps_s.tile([128, 1024], BF16, tag="pss",
                                name=f"psx_{b}_{m}_{g}")
                for kk in range(3):
                    kb = 3 * g + kk
                    nc.tensor.transpose(pst[:, 128 * kk:128 * (kk + 1)],
                                        xc[:, 128 * kb:128 * (kb + 1)],
                                        id16[:])
                dst = xTv[:, 3 * g:3 * g + 3, 128 * m:128 * (m + 1)]
                nc.vector.tensor_copy(dst, pst[:, 0:384].rearrange(
                    "p (kk t) -> p kk t", t=128))

        def phase_d(b, attn, wp01, early=False):
            # out projection for batch b (emitted after the NEXT batch's QKV
            # phase so the normalize-chain tail hides under independent PE work)
            def emit_group(ni, m, py):
                n0 = ni * NP
                yo = sbyo.tile([128, NP], F32, tag="yo")
                nc.vector.tensor_tensor(out=yo[:], in0=py,
                                        in1=bp_b[:, n0:n0 + NP], op=ADD)
                nc.sync.dma_start(
                    out_d[b, 128 * m:128 * (m + 1), n0:n0 + NP], yo[:])

            wpvs = [w[:].rearrange("p (kt c) -> p kt c", c=NP) for w in wp01]
            skip = set()
            if early:
                # last batch: no later QKV phase hides the normalize-chain
                # tail, so open 4 psum groups on kt 0..6 (they only need
                # heads <= 12, ready ~3 heads early) and close kt 7..8 after
                pys = []
                for m in range(4):
                    if m < 2:
                        psum = ps_s.tile([128, 512], F32, tag="pss")
                    else:
                        psum = ps_sc.tile([128, 1024], F32, tag="sc")
                    pys.append(psum[:, 0:NP])
                    for kt in range(7):
                        nc.tensor.matmul(pys[m],
                                         attn[kt][:, 128 * m:128 * (m + 1)],
                                         wpvs[0][:, kt, :], start=(kt == 0),
                                         stop=False)
                for m in range(4):
                    for kt in range(7, KT_O):
                        nc.tensor.matmul(pys[m],
                                         attn[kt][:, 128 * m:128 * (m + 1)],
                                         wpvs[0][:, kt, :], start=False,
                                         stop=(kt == KT_O - 1))
                    emit_group(0, m, pys[m])
                    skip.add((0, m))
            for ni in range(H // NP):
                for m in range(TT):
                    if (ni, m) in skip:
                        continue
                    psum = ps_s.tile([128, 512], F32, tag="pss")
                    py = psum[:, 0:NP]
                    for kt in range(KT_O):
                        nc.tensor.matmul(py, attn[kt][:, 128 * m:128 * (m + 1)],
                                         wpvs[ni][:, kt, :], start=(kt == 0),
                                         stop=(kt == KT_O - 1))
                    emit_group(ni, m, py)

        def prefetch_wp(b):
            wp01 = []
            for ni in range(3):
                wpch = sbwp.tile([128, KT_O * NP], BF16, tag="wp",
                                 name=f"wp{b}_{ni}")
                nc.sync.dma_start(
                    wpch[:].rearrange("p (kt c) -> p kt c", c=NP),
                    wp_d[:, ni * NP:(ni + 1) * NP].rearrange(
                        "(kt p) c -> p kt c", p=128))
                wp01.append(wpch)
            return wp01

        next_xTv = None
        pending_d = None
        for b in range(n_batch):
            # ---------------- phase A: load x, transpose to xT ----------------
            if next_xTv is None:
                xT = sbx.tile([128, 9 * S], BF16, tag="xT", name=f"xT_{b}")
                xTv = xT[:].rearrange("p (kb t) -> p kb t", t=S)
                for m in range(TT):
                    phase_a_tile(b, xTv, m)
            else:
                xTv = next_xTv
            next_xTv = None
            if pending_d is not None:
                wp01_prev = prefetch_wp(b - 1)

            # ---------------- phase B: QKV projection ----------------
            q_sb = [sbqk.tile([128, H], BF16, tag=f"q{m}", name=f"q{m}_{b}") for m in range(TT)]
            k_sb = [sbqk.tile([128, H], BF16, tag=f"k{m}", name=f"k{m}_{b}") for m in range(TT)]
            vaug = [sbv.tile([128, 97 * NH], BF16, tag=f"v{m}", name=f"v{m}_{b}") for m in range(TT)]
            stats = sbs.tile([128, 2 * NH * TT], F32, tag="stats",
                             name=f"stats_{b}")
            for m in range(TT):
                nc.vector.tensor_copy(
                    vaug[m][:].rearrange("p (h c) -> p h c", c=97)[:, :, 72:97],
                    zo[:].unsqueeze(1).broadcast_to([128, NH, 25]))
            for tens in range(3):  # 0=q, 1=k, 2=v
                for (coff, chw) in CHUNKS:
                    c0 = tens * H + coff
                    nhh = chw // HD
                    h0 = coff // HD
                    wch = sbw.tile([128, 9 * 432], BF16, tag="w", name=f"w{b}_{tens}_{coff}")
                    nc.sync.dma_start(
                        wch[:].rearrange("p (kb c) -> p kb c", c=432)[:, :, 0:chw],
                        wqkv_d[:, c0:c0 + chw].rearrange("(kb p) c -> p kb c", p=128))
                    wv = wch[:].rearrange("p (kb c) -> p kb c", c=432)
                    for m in range(TT):
                        psum = ps_s.tile([128, 512], F32, tag="pss")
                        pr = psum[:, 0:chw]
                        for kb in range(9):
                            nc.tensor.matmul(pr, xTv[:, kb, 128 * m:128 * (m + 1)],
                                             wv[:, kb, 0:chw], start=(kb == 0),
                                             stop=(kb == 8))
                        if tens == 2:  # v -> vaug strided (+bias)
                            dst = vaug[m][:].rearrange("p (h c) -> p h c", c=97)[
                                :, h0:h0 + nhh, 0:72]
                            nc.vector.tensor_tensor(
                                out=dst, in0=pr.rearrange("p (h c) -> p h c", c=HD),
                                in1=bias_b[:, c0:c0 + chw].rearrange(
                                    "p (h c) -> p h c", c=HD), op=ADD)
                        else:
                            dsttile = q_sb[m] if tens == 0 else k_sb[m]
                            nc.vector.tensor_tensor(
                                out=dsttile[:, coff:coff + chw], in0=pr,
                                in1=bias_b[:, c0:c0 + chw], op=ADD)
                            qsq = sbt.tile([128, 432], F32, tag="qsq")
                            nc.scalar.activation(
                                qsq[:, 0:chw], dsttile[:, coff:coff + chw],
                                AF.Square)
                            so = 2 * NH * m + NH * tens + h0
                            nc.vector.reduce_sum(
                                stats[:, so:so + nhh],
                                qsq[:, 0:chw].rearrange("p (h c) -> p h c", c=HD),
                                axis=mybir.AxisListType.X)
            # rinv (batched: one sqrt + one reciprocal per batch) + apply
            rms = sbs.tile([128, 2 * NH * TT], F32, tag="rms", name=f"rms_{b}")
            nc.scalar.activation(rms[:], stats[:], AF.Sqrt,
                                 scale=1.0 / HD, bias=eps_t[:])
            nc.vector.reciprocal_approx_fast(rms[:], rms[:])
            for m in range(TT):
                for tens in range(2):
                    dsttile = q_sb[m] if tens == 0 else k_sb[m]
                    so = 2 * NH * m + NH * tens
                    rb3 = rms[:, so:so + NH].unsqueeze(2) \
                        .broadcast_to([128, NH, HD])
                    dv = dsttile[:].rearrange("p (h c) -> p h c", c=HD)
                    nc.vector.tensor_tensor(out=dv, in0=dv, in1=rb3, op=MUL)

            if pending_d is not None:
                phase_d(pending_d[0], pending_d[1], wp01_prev)
                pending_d = None

            # ---------------- phase C: attention per head ----------------
            # attn rows packed dense: head h at rows 72h..72h+71 (9 K-tiles
            # for the out-proj, written by partition-shifting SBUF DMAs)
            attn = [sba.tile([128, S], BF16, tag=f"a{t}", name=f"a{t}_{b}") for t in range(KT_O)]
            posbs = {}
            dn_d = rcp_d = rcg = rco = None

            def build_qkT(h, tens):
                src = q_sb if tens == 0 else k_sb
                dst = sbqt.tile([HD, S], BF16, tag=("qT" if tens == 0 else "kT"),
                                name=f"{'qk'[tens]}T_{b}_{h}")
                for g in range(2):  # 4 tok-tiles per psum group
                    pst = ps_s.tile([128, 1024], BF16, tag="pss",
                                    name=f"pst_{b}_{h}_{tens}_{g}")
                    for mm in range(4):
                        m = 4 * g + mm
                        nc.tensor.transpose(
                            pst[0:HD, 128 * mm:128 * (mm + 1)],
                            src[m][:, HD * h:HD * (h + 1)], id16[:])
                    nc.vector.tensor_copy(dst[:, 512 * g:512 * (g + 1)],
                                          pst[0:HD, 0:512])
                if tens == 1:
                    nc.vector.tensor_scalar_mul(dst[:], dst[:], gqk[:])
                return dst

            nxt = (build_qkT(0, 0), build_qkT(0, 1))
            for h in range(NH):
                qT, kT = nxt
                po = ps_pv.tile([128, 1024], F32, tag="pv")
                # software-pipelined: scores for jt+1 issue before PV of jt so
                # the in-order PE stream never stalls on exp(jt)
                def scores(jt):
                    pss = ps_sc.tile([128, 1024], F32, tag="sc",
                                     name=f"pss_{b}_{h}_{jt}")
                    for ih in range(2):
                        nc.tensor.matmul(pss[:, 512 * ih:512 * (ih + 1)],
                                         kT[:, 128 * jt:128 * (jt + 1)],
                                         qT[:, 512 * ih:512 * (ih + 1)],
                                         start=True, stop=True)
                    return pss
                pss_cur = scores(0)
                for jt in range(TT):
                    eT = sbe.tile([128, S], BF16, tag="eT")
                    nc.scalar.activation(eT[:], pss_cur[:], AF.Exp, scale=SCALE)
                    if jt + 1 < TT:
                        pss_cur = scores(jt + 1)
                    # prefetch next head's transposes into exp-wait bubbles
                    if h + 1 < NH and jt == 2:
                        nq = build_qkT(h + 1, 0)
                    elif h + 1 < NH and jt == 5:
                        nxt = (nq, build_qkT(h + 1, 1))
                    elif b + 1 < n_batch and 8 <= h and jt == 7:
                        # overlap next batch's x-load/transpose with attention
                        if h == 8:
                            nxT = sbx.tile([128, 9 * S], BF16, tag="xT",
                                           name=f"xT_{b + 1}")
                            next_xTv = nxT[:].rearrange("p (kb t) -> p kb t",
                                                        t=S)
                        phase_a_tile(b + 1, next_xTv, h - 8)
                    for ih in range(2):
                        nc.tensor.matmul(po[0:97, 512 * ih:512 * (ih + 1)],
                                         vaug[jt][:, 97 * h:97 * h + 97],
                                         eT[:, 512 * ih:512 * (ih + 1)],
                                         start=(jt == 0), stop=(jt == TT - 1))
                # evacuate PV accumulator (frees the psum bank fast), then a
                # fully per-head normalize chain with only TWO DMA hops: the
                # f32 denominator row bounces to DRAM, broadcast-reads back as
                # [72, S], and reciprocal_approx_fast runs on the broadcast
                # (full lanes) before the in-place multiply.
                j4 = h % 4
                if j4 == 0:
                    dn_d = dpool.tile([4, S], F32, tag="dn", name=f"dn_{b}_{h}")
                posb = sbpo.tile([97, S], BF16, tag=f"posb{h % 4}",
                                 name=f"posb_{b}_{h}")
                posbs[h] = posb
                nc.vector.tensor_copy(posb[:], po[0:97, :])
                rb = sbr.tile([HD, S], F32, tag="rb", name=f"rb_{b}_{h}")
                if h == NH - 1:
                    # last head is the exposed critical path: broadcast the
                    # denominator row on the PE (K=1 matmul into the freed PV
                    # psum slot) instead of two ~4us DMA hops
                    rb_ps = ps_pv.tile([128, 1024], F32, tag="pv",
                                       name=f"rbps_{b}")
                    for ih in range(2):
                        nc.tensor.matmul(rb_ps[0:HD, 512 * ih:512 * (ih + 1)],
                                         ones72[96:97, :],
                                         posb[96:97, 512 * ih:512 * (ih + 1)],
                                         start=True, stop=True,
                                         tile_position=(96, 0))
                    nc.vector.reciprocal_approx_fast(rb[:], rb_ps[0:HD, :])
                else:
                    nc.gpsimd.dma_start(dn_d[j4:j4 + 1, :], posb[96:97, :])
                    nc.sync.dma_start(
                        rb[:], dn_d[j4:j4 + 1, :].broadcast_to([HD, S]))
                    nc.vector.reciprocal_approx_fast(rb[:], rb[:])
                eng = nc.gpsimd if (h % 2 == 0) else nc.vector
                eng.tensor_tensor(out=posb[0:HD, :], in0=posb[0:HD, :],
                                  in1=rb[:], op=MUL)
                # dense repack: head h -> attn rows 72h..72h+71 (DMA shifts
                # partitions; compute stayed 32-aligned in posb)
                r0 = HD * h
                t0, off = divmod(r0, 128)
                ln = min(128 - off, HD)
                nc.sync.dma_start(attn[t0][off:off + ln, :], posb[0:ln, :])
                if ln < HD:
                    nc.sync.dma_start(attn[t0 + 1][0:HD - ln, :],
                                      posb[ln:HD, :])
                if b == n_batch - 1 and h == NH - 2:
                    # last batch: its out-proj follows immediately, so get the
                    # first weight chunks moving before the final chain
                    wp01_last = prefetch_wp(b)

            if b == n_batch - 1:
                phase_d(b, attn, wp01_last, early=True)
            else:
                pending_d = (b, attn)


_NC_CACHE = {}


def _get_nc(n_batch=B_LOCAL):
    if n_batch not in _NC_CACHE:
        _NC_CACHE[n_batch] = build_nc(n_batch)
    return _NC_CACHE[n_batch]


def prep_inputs(w_qkv, b_qkv, q_gamma, k_gamma, w_proj, b_proj, **_ignored):
    """Host-side layout prep shared by all cores (non-x inputs)."""
    w_qkv = np.asarray(w_qkv, np.float32)
    b_qkv = np.asarray(b_qkv, np.float32)
    q_gamma = np.asarray(q_gamma, np.float32)
    k_gamma = np.asarray(k_gamma, np.float32)
    w_proj = np.asarray(w_proj, np.float32)
    b_proj = np.asarray(b_proj, np.float32)

    wqkvt = np.ascontiguousarray(w_qkv.T).astype(ml_dtypes.bfloat16)  # [H, 3H]
    biasb = np.ascontiguousarray(
        np.broadcast_to(b_qkv, (128, 3 * H))).astype(ml_dtypes.bfloat16)
    gqk = np.ascontiguousarray((q_gamma * k_gamma).reshape(HD, 1))
    wprojt = np.ascontiguousarray(w_proj.T).astype(ml_dtypes.bfloat16)
    bprojb = np.ascontiguousarray(np.broadcast_to(b_proj, (128, H)))
    return {
        "wqkvt": wqkvt, "biasb": biasb, "gqk": gqk,
        "wprojt": wprojt, "bprojb": bprojb,
    }


def run(inputs, trace=False, n_batch=B_LOCAL, n_cores=N_CORES, **run_kwargs):
    """Shard inputs, run SPMD, gather output. Returns (out [B,S,H], results)."""
    x = np.asarray(inputs["x"], np.float32)
    common = prep_inputs(**{k: v for k, v in inputs.items() if k != "x"})
    nc = _get_nc(n_batch)
    in_maps = []
    for c in range(n_cores):
        m = dict(common)
        m["x"] = np.ascontiguousarray(x[c * n_batch:(c + 1) * n_batch])
        in_maps.append(m)
    res = run_bass_kernel_spmd(nc, in_maps, core_ids=list(range(n_cores)),
                               trace=trace, **run_kwargs)
    out = np.concatenate([res.results[c]["out"] for c in range(n_cores)],
                         axis=0)
    return out, res


def kernel(**inputs) -> np.ndarray:
    out, _ = run(inputs)
    return out

